# revision 11
# baseline (speedup 1.0000x reference)
"""Trainium2 Bass kernel for DigitCapsules dynamic routing (v2).

Problem: u [256, 2048, 8] f32, W [1, 2048, 10, 16, 8] f32
  u_hat = einsum('pcoi,bpi->bpco', W[0], u)
  3 routing iterations (softmax over c, weighted sum over p, squash,
  agreement update) -> v [256, 10, 16] f32.

Strategy (8 cores data-parallel over batch, 32 batch elems per core,
4 groups of 8 batch elems per core, pipelined):
  - Partition layout: SBUF partition = (p_local 16, b 8); PE contraction
    K = (p_local 16, i 8) = 128 via block-diagonal stationary u_bd.
  - uhat [128, slab, o, c] bf16; PSUM evacuated in 9-slab batches
    (3 banks) by ACT/Pool copies that do the (c,o)->(o,c) transpose.
  - b-logit linearity: b_t = sum_o uhat * (v1+..+v_{t-1}), so the
    routing keeps only an accumulated V per group (on all 128
    partitions) and never stores per-iteration b-state.
  - The p-reduction (s-step) uses a replicated block-diag ones
    stationary with M=128 so s/v/squash live replicated on all 128
    partitions -> no DMA broadcasts inside the routing loop.
  - Scalar engine only ever uses {Copy, Exp, Ln} which share one
    activation table (rsqrt computed as exp(-0.5*ln(x))) -> a single
    ACT table load for the whole kernel.
  - Elementwise work split across DVE (tensor_tensor, 2x bf16 mode)
    and Pool/GpSimd (scalar_tensor_tensor at 0.6 efficiency); softmax
    Z-reduce and cw-scale on Pool; PSUM evac split ACT/Pool.
"""

import numpy as np
import ml_dtypes

bf16 = ml_dtypes.bfloat16

# Problem constants (fixed by the problem spec; do not read spec.json here)
B, P, C, O, IN = 256, 2048, 10, 16, 8
NCORES = 8
B_LOC = B // NCORES          # 32 batch elems per core
BT = 8                       # batch elems per group
NGROUP = B_LOC // BT         # 4 groups per core
PSLAB = 16                   # p-values per slab
NSLAB = P // PSLAB           # 128 slabs
CO = C * O                   # 160
ROUTING_ITERS = 3
EPS = 1e-9

CHB = 16    # slabs per u_bd DMA chunk
EVB = 3     # slabs per PSUM evacuation batch (1 bank)
CHS = 32    # slabs per routing compute chunk
SMM = 2     # slabs per s-step matmul (N = SMM*CO = 320)


def _host_prep(u_core, W0, nslab=NSLAB, ngroup=NGROUP):
    """Build host-side reordered (k-major, contiguous-DMA) arrays."""
    # w_k[p*8+i, s, c*16+o] = W0[16s+p, c, o, i]
    w = W0.reshape(nslab, PSLAB, C, O, IN)
    w_k = np.ascontiguousarray(
        w.transpose(1, 4, 0, 2, 3).reshape(PSLAB * IN, nslab, CO)
    ).astype(bf16)

    # x[g, b, s, p, i] = u_core[g*8 + b, 16s+p, i]
    x = u_core.reshape(ngroup, BT, nslab, PSLAB, IN)

    # ubd_k[g, p*8+i, s, p'*8+b] = x[g,b,s,p,i] * (p == p')
    xt = x.transpose(0, 3, 4, 2, 1)  # [g, p, i, s, b]
    ubd_k = np.zeros((ngroup, PSLAB, IN, nslab, PSLAB, BT), dtype=bf16)
    for p in range(PSLAB):
        ubd_k[:, p, :, :, p, :] = xt[:, p]
    ubd_k = ubd_k.reshape(ngroup, PSLAB * IN, nslab, PSLAB * BT)

    # ut_k[p*8+i, s, g*8+b] = x[g,b,s,p,i] -- dense stationary for the
    # s1 sweep covering ALL groups (M = ngroup*BT)
    ut_k = np.ascontiguousarray(
        x.transpose(3, 4, 2, 0, 1).reshape(PSLAB * IN, nslab, ngroup * BT)
    ).astype(bf16)

    # ones2[p*8+b, p'*8+b'] = (b == b') -- replicated p-sum stationary
    ones2 = np.zeros((PSLAB * BT, PSLAB * BT), dtype=bf16)
    for p in range(PSLAB):
        for p2 in range(PSLAB):
            for b in range(BT):
                ones2[p * BT + b, p2 * BT + b] = 1.0
    return {"w_k": w_k, "ubd_k": ubd_k, "ut_k": ut_k, "ones2": ones2}


def build(nc, tc, ctx, nslab=NSLAB, ngroup=NGROUP):
    """Emit the kernel IR."""
    import concourse.bass as bass
    from concourse import mybir

    f32 = mybir.dt.float32
    bf = mybir.dt.bfloat16
    Alu = mybir.AluOpType
    Act = mybir.ActivationFunctionType
    Ax = mybir.AxisListType

    b_loc = ngroup * BT
    chb = min(CHB, nslab)
    chs = min(CHS, nslab)
    nchunk = max(1, nslab // chs)

    # ---- DRAM parameters ----
    w_dram = nc.dram_tensor(
        "w_k", [PSLAB * IN, nslab, CO], bf, kind="ExternalInput").ap()
    ubd_dram = nc.dram_tensor(
        "ubd_k", [ngroup, PSLAB * IN, nslab, PSLAB * BT], bf,
        kind="ExternalInput").ap()
    ut_dram = nc.dram_tensor(
        "ut_k", [PSLAB * IN, nslab, ngroup * BT], bf,
        kind="ExternalInput").ap()
    ones2_dram = nc.dram_tensor(
        "ones2", [PSLAB * BT, PSLAB * BT], bf, kind="ExternalInput").ap()
    vout_dram = nc.dram_tensor("v_out", [b_loc, CO], f32,
                               kind="ExternalOutput").ap()
    vscr_dram = nc.dram_tensor("v_scratch", [ngroup * BT, O * C], bf).ap()

    # ---- pools ----
    consts = ctx.enter_context(tc.tile_pool(name="consts", bufs=1))
    ubdpool = ctx.enter_context(tc.tile_pool(name="ubdpool", bufs=3))
    uhatpool = ctx.enter_context(tc.tile_pool(name="uhat", bufs=2))
    upsum = ctx.enter_context(tc.tile_pool(name="upsum", bufs=5, space="PSUM"))
    spsum = ctx.enter_context(tc.tile_pool(name="spsum", bufs=2, space="PSUM"))
    tmp = ctx.enter_context(tc.tile_pool(name="tmp", bufs=2))
    state = ctx.enter_context(tc.tile_pool(name="state", bufs=2))
    small = ctx.enter_context(tc.tile_pool(name="small", bufs=2))
    vpool = ctx.enter_context(tc.tile_pool(name="vpool", bufs=2))

    ones2_sb = consts.tile([PSLAB * BT, PSLAB * BT], bf)
    nc.sync.dma_start(out=ones2_sb[:], in_=ones2_dram)

    # resident W: whole tensor, four contiguous quarters on the ACT queue
    wall = consts.tile([PSLAB * IN, nslab, CO], bf)
    h = max(1, nslab // 4)
    for j in range(0, nslab, h):
        nc.scalar.dma_start(out=wall[:, j:j + h, :], in_=w_dram[:, j:j + h, :])

    ut_res = consts.tile([PSLAB * IN, nslab, ngroup * BT], bf)
    nc.sync.dma_start(out=ut_res[:], in_=ut_dram)

    # V1 broadcast tiles, one per group
    V1 = [consts.tile([PSLAB * BT, O, C], bf, tag=f"v1g{g}",
                      name=f"v1g{g}")
          for g in range(ngroup)]

    def bcast_ap(ap, insert_pos, size):
        """Insert a stride-0 dim of `size` at free-dim position insert_pos."""
        new = list(ap.ap)
        new.insert(insert_pos, [0, size])
        return bass.AP(tensor=ap.tensor, offset=ap.offset, ap=new)

    def squash_fac(s_sb, n):
        """s_sb: [n, C, O] f32 -> fac [n, C] f32 (squash scale factor).
        fac = nrm/(1+nrm) * rsqrt(nrm+eps), rsqrt via exp(-0.5*ln)."""
        sq = small.tile([n, C, O], f32, tag="sq")
        nc.scalar.activation(sq[:], s_sb[:], Act.Square)
        nrm = small.tile([n, C], f32, tag="nrm")
        nc.vector.tensor_reduce(out=nrm[:], in_=sq[:], axis=Ax.X, op=Alu.add)
        d1 = small.tile([n, C], f32, tag="d1")
        nc.vector.tensor_scalar_add(d1[:], nrm[:], 1.0)
        r1 = small.tile([n, C], f32, tag="r1s")
        nc.vector.reciprocal(r1[:], d1[:])
        se = small.tile([n, C], f32, tag="se")
        nc.vector.tensor_scalar_add(se[:], nrm[:], EPS)
        lnse = small.tile([n, C], f32, tag="lnse")
        nc.scalar.activation(lnse[:], se[:], Act.Ln)
        r2 = small.tile([n, C], f32, tag="r2s")
        nc.scalar.activation(r2[:], lnse[:], Act.Exp, scale=-0.5)
        f1 = small.tile([n, C], f32, tag="f1")
        nc.vector.tensor_tensor(out=f1[:], in0=nrm[:], in1=r1[:], op=Alu.mult)
        fac = small.tile([n, C], f32, tag="fac")
        nc.vector.tensor_tensor(out=fac[:], in0=f1[:], in1=r2[:], op=Alu.mult)
        return fac

    # ---------- s1 sweep: one accumulation for ALL groups (M=32) ----------
    s1_ps = spsum.tile([ngroup * BT, CO], f32, tag="sps", name="s1ps")
    for s in range(nslab):
        nc.tensor.matmul(
            out=s1_ps[:], lhsT=ut_res[:, s, :], rhs=wall[:, s, :],
            start=(s == 0), stop=(s == nslab - 1))
    s1_sb = small.tile([ngroup * BT, C, O], f32, tag="s1sb")
    nc.scalar.mul(s1_sb[:].rearrange("n c o -> n (c o)"), s1_ps[:], 1.0 / C)
    fac1 = squash_fac(s1_sb, ngroup * BT)
    v1_bf = small.tile([ngroup * BT, O * C], bf, tag="v1bf")
    nc.vector.tensor_tensor(
        out=v1_bf[:].rearrange("n (o c) -> n c o", o=O),
        in0=s1_sb[:], in1=bcast_ap(fac1[:], 2, O), op=Alu.mult)
    nc.sync.dma_start(out=vscr_dram, in_=v1_bf[:])
    for g in range(ngroup):
        src = bass.AP(
            tensor=vscr_dram.tensor,
            offset=vscr_dram.offset + g * BT * CO,
            ap=[[0, PSLAB], [O * C, BT], [1, O * C]])
        nc.sync.dma_start(out=V1[g][:], in_=src)

    # ---------- phase A: u_hat materialization ----------
    def phase_a(g):
        uhat = uhatpool.tile([128, nslab, O, C], bf, tag="uhat",
                             name=f"uhat{g}")
        ubs = {}

        def get_ub(ci):
            if ci not in ubs:
                ub = ubdpool.tile([PSLAB * IN, chb, PSLAB * BT], bf,
                                  tag="ubd")
                nc.sync.dma_start(
                    out=ub[:], in_=ubd_dram[g, :, ci * chb:(ci + 1) * chb, :])
                ubs[ci] = ub
            return ubs[ci]

        s0 = 0
        while s0 < nslab:
            nb = min(EVB, nslab - s0)
            ps = upsum.tile([128, EVB, CO], f32, tag="ups")
            get_ub(s0 // chb)
            if (s0 + nb - 1) // chb != s0 // chb:
                get_ub((s0 + nb - 1) // chb)
            # prefetch next chunk
            nxt = (s0 + nb) // chb
            if nxt * chb < nslab:
                get_ub(nxt)
            for q in range(nb):
                sl = s0 + q
                ub = ubs[sl // chb]
                nc.tensor.matmul(
                    out=ps[:, q, :], lhsT=ub[:, sl % chb, :],
                    rhs=wall[:, sl, :], start=True, stop=True)
            # evacuate nb slabs, transposing (c,o) -> (o,c)
            src = ps[:, 0:nb, :].rearrange("p s (c o) -> p s o c", c=C)
            dst = uhat[:, s0:s0 + nb, :, :]
            nc.scalar.copy(dst, src)
            s0 += nb
        return uhat

    # ---------- routing ----------
    # number of G-premul chunks handed to Pool per (g,it) unit, cycled
    GPRE_POOL = [1, 1, 1, 1, 1, 1, 1, 1]

    def route(g, uhat):
        Vcur = V1[g]
        for it in range(1, ROUTING_ITERS):
            git = g * (ROUTING_ITERS - 1) + (it - 1)
            npool = GPRE_POOL[git % len(GPRE_POOL)]
            # G-step: logits bst[p,b][s,c] = sum_o uhat * Vcur
            bst = state.tile([128, nslab, C], bf, tag="bst")
            for ch in range(nchunk):
                sl = slice(ch * chs, (ch + 1) * chs)
                t2 = tmp.tile([128, chs, O, C], bf, tag="tt")
                if ch < npool:
                    # Pool needs <=3D APs: flatten (o,c) -> 160 contiguous
                    vflat = Vcur[:].rearrange("p o c -> p (o c)")
                    nc.gpsimd.tensor_tensor(
                        out=t2[:].rearrange("p s o c -> p s (o c)"),
                        in0=uhat[:, sl, :, :].rearrange(
                            "p s o c -> p s (o c)"),
                        in1=bcast_ap(vflat, 1, chs), op=Alu.mult)
                else:
                    nc.vector.tensor_tensor(
                        out=t2[:], in0=uhat[:, sl, :, :],
                        in1=bcast_ap(Vcur[:], 1, chs), op=Alu.mult)
                r1 = tmp.tile([128, chs, O // 2, C], bf, tag="r1t")
                nc.vector.tensor_tensor(
                    out=r1[:], in0=t2[:, :, 0:O // 2, :],
                    in1=t2[:, :, O // 2:O, :], op=Alu.add)
                r2 = tmp.tile([128, chs, O // 4, C], bf, tag="r2t")
                nc.vector.tensor_tensor(
                    out=r2[:], in0=r1[:, :, 0:O // 4, :],
                    in1=r1[:, :, O // 4:O // 2, :], op=Alu.add)
                r3 = tmp.tile([128, chs, 2, C], bf, tag="r3t")
                nc.gpsimd.tensor_tensor(
                    out=r3[:], in0=r2[:, :, 0:2, :],
                    in1=r2[:, :, 2:4, :], op=Alu.add)
                nc.gpsimd.tensor_tensor(
                    out=bst[:, sl, :], in0=r3[:, :, 0, :],
                    in1=r3[:, :, 1, :], op=Alu.add)
            # softmax over c
            expt = state.tile([128, nslab, C], bf, tag="expt")
            nc.scalar.activation(expt[:], bst[:], Act.Exp)
            Z = state.tile([128, nslab], f32, tag="Z")
            nc.vector.tensor_reduce(out=Z[:], in_=expt[:], axis=Ax.X,
                                    op=Alu.add)
            rz = state.tile([128, nslab], f32, tag="rz")
            nc.vector.reciprocal(rz[:], Z[:])
            cw = state.tile([128, nslab, C], bf, tag="cw")
            nc.gpsimd.tensor_tensor(
                out=cw[:], in0=expt[:],
                in1=bcast_ap(rz[:], 2, C), op=Alu.mult)
            # s-step: premul + replicated block-diag ones reduction
            s_ps = spsum.tile([128, SMM, O, C], f32, tag="sps", name="sps")
            for ch in range(nchunk):
                sl = slice(ch * chs, (ch + 1) * chs)
                t1 = tmp.tile([128, chs, O, C], bf, tag="tt")
                cwb = bcast_ap(cw[:, sl, :], 2, O)
                nc.vector.tensor_tensor(
                    out=t1[:], in0=uhat[:, sl, :, :], in1=cwb, op=Alu.mult)
                for k in range(chs // SMM):
                    s_idx = ch * chs + k * SMM
                    nc.tensor.matmul(
                        out=s_ps[:], lhsT=ones2_sb[:],
                        rhs=t1[:, k * SMM:(k + 1) * SMM, :, :],
                        start=(s_idx == 0), stop=(s_idx == nslab - SMM))
            # collect the SMM slab-positions; each is (o,c) ordered
            s_rw = small.tile([128, SMM, O, C], f32, tag="s_rw")
            nc.scalar.copy(s_rw[:], s_ps[:])
            s_sb = small.tile([128, C, O], f32, tag="s_sb")
            nc.vector.tensor_tensor(
                out=s_sb[:],
                in0=s_rw[:, 0, :, :].rearrange("p o c -> p c o"),
                in1=s_rw[:, 1, :, :].rearrange("p o c -> p c o"),
                op=Alu.add)
            fac = squash_fac(s_sb, 128)
            if it == ROUTING_ITERS - 1:
                vfin = small.tile([128, C, O], f32, tag="vfin")
                nc.vector.tensor_tensor(
                    out=vfin[:], in0=s_sb[:], in1=bcast_ap(fac[:], 2, O),
                    op=Alu.mult)
                nc.sync.dma_start(
                    out=vout_dram[g * BT:(g + 1) * BT, :],
                    in_=vfin[0:BT, :, :].rearrange("n c o -> n (c o)"))
            else:
                v_bf = small.tile([128, O * C], bf, tag="vbf")
                nc.vector.tensor_tensor(
                    out=v_bf[:].rearrange("p (o c) -> p c o", o=O),
                    in0=s_sb[:], in1=bcast_ap(fac[:], 2, O), op=Alu.mult)
                Vnext = vpool.tile([128, O, C], bf, tag="vacc")
                nc.vector.tensor_tensor(
                    out=Vnext[:], in0=Vcur[:],
                    in1=v_bf[:].rearrange("p (o c) -> p o c", o=O),
                    op=Alu.add)
                Vcur = Vnext

    # Sequential emission; the Tile scheduler overlaps group k+1's phase A
    # with group k's routing (uhat pool has 2 buffers).
    for g in range(ngroup):
        route(g, phase_a(g))


def make_inputs_per_core(u, W):
    """Full inputs -> list of 8 in_maps."""
    W0 = np.asarray(W, dtype=np.float32)[0]
    u = np.asarray(u, dtype=np.float32)
    in_maps = []
    for c in range(NCORES):
        u_core = u[c * B_LOC:(c + 1) * B_LOC]
        in_maps.append(_host_prep(u_core, W0))
    return in_maps


def numpy_model(u_core, W0):
    """f32 numpy model of the routing (for small-scale checks)."""
    u_hat = np.einsum('pcoi,bpi->bpco', W0, u_core)
    b = np.zeros(u_hat.shape[:3], dtype=np.float32)
    v = None
    for _ in range(ROUTING_ITERS):
        e = np.exp(b - b.max(axis=2, keepdims=True))
        c = e / e.sum(axis=2, keepdims=True)
        s = np.einsum('bpc,bpco->bco', c, u_hat)
        sq = (s * s).sum(-1, keepdims=True)
        v = (sq / (1 + sq)) * s / np.sqrt(sq + EPS)
        b = b + np.einsum('bpco,bco->bpc', u_hat, v)
    return v


_COMPILED = {}


def _get_compiled():
    if "nc" in _COMPILED:
        return _COMPILED["nc"]
    from contextlib import ExitStack
    import concourse.tile as tile
    from concourse import bacc

    nc = bacc.Bacc("TRN2", target_bir_lowering=False, debug=False,
                   num_devices=NCORES)
    with tile.TileContext(nc) as tc:
        with ExitStack() as ctx:
            build(nc, tc, ctx)
    nc.compile()
    _COMPILED["nc"] = nc
    return nc


def kernel(u, W):
    """Full-input entry point: u [256,2048,8] f32, W [1,2048,10,16,8] f32
    -> v [256, 10, 16] f32."""
    from concourse.bass_utils import run_bass_kernel_spmd

    nc = _get_compiled()
    in_maps = make_inputs_per_core(u, W)
    res = run_bass_kernel_spmd(nc, in_maps, core_ids=list(range(NCORES)))
    outs = [res.results[c]["v_out"] for c in range(NCORES)]
    v = np.concatenate(outs, axis=0).reshape(B, C, O).astype(np.float32)
    return v
